# revision 13
# baseline (speedup 1.0000x reference)
"""Trainium2 Bass kernel for MockBitNetLayer:

    scale = mean(|W|, axis=1)            # [O, 1]
    y = x @ (sign(W) * scale).T + bias   # [T, O]

Strategy (column-parallel over 8 NeuronCores), v6:
  - Each core owns an O/8 = 2048-column shard of W.T and bias; x is
    shared.  Host-side input marshaling (transpose + dtype cast + tile
    layout) is done in numpy during sharding; all model arithmetic
    (sign, |W| mean, matmul, scale/bias) runs on device.
  - Precision split over the contraction: the first NK8*128 rows of x
    in fp8e4 (consumed by DoubleRow MMs, 2 k-tiles per 216 ns slot),
    the rest in fp16 (1 k-tile per slot).  NK8=18 measures 1.989e-2
    against the fp32 reference (tolerance 2e-2), predicted exactly by
    a host-side numpy simulation of the quantization chain.
  - W arrives twice: k-major fp8e5 (sign path; values that would round
    to zero/denormal are host-pinned to +-2^-14 so sign() is exact) and
    o-major fp16 (scale path).
  - Engine/queue split (v5 traces showed sign-slab DMAs pacing the
    whole warmup, and the gpsimd SWDGE ring sustaining only ~80 GB/s):
      sync   : 32 full-width contiguous W.T slab DMAs (256 KB each),
               then the o-major W + bias for the scale path
      scalar : x chunk DMAs (c0/c1 pre-issued), then 64 sign ops
               (two 1024-wide halves per slab, oh0 halves first so the
               first MM blocks' weights appear earliest)
      vector : |W| row reduces (f16, 2x DVE rate) + 1/K, then all psum
               evictions (tensor_scalar psum*scale+bias -> f16)
      gpsimd : y write-out DMAs only
      tensor : nothing but the MM stream
  - Chunk 0 runs (ob0..3)/(ob4..7) interleaved across 8 PSUM banks,
    halving the fresh-slab consumption rate while the sign pipeline
    warms; later chunks roll 2-o-tile psum groups (4 in flight).
  - y is written as f16 (halves output traffic); the last chunk's y
    DMAs go out on the idle sync/scalar rings to shrink the tail.
"""

import os
import sys

for _p in ("/opt/trn_rl_repo", "/root/.axon_site/_ro/trn_rl_repo"):
    if os.path.isdir(_p) and _p not in sys.path:
        sys.path.insert(0, _p)

import numpy as np
import ml_dtypes

import concourse.bacc as bacc
import concourse.mybir as mybir
import concourse.tile as tile
from concourse.bass import ds
from concourse.bass_utils import run_bass_kernel_spmd

P = 128
N_CORES = 8

T_FULL = 8192
K_FULL = 4096
O_FULL = 16384

NK8 = int(os.environ.get("NK8T", "18"))  # fp8 k-tiles (even, 0..32)
TCH = 512
SW = 16  # W.T slab stage ring depth


def build_kernel_body(tc, xt8, xt16, wt, w16, b, yt, T, K, O, nk8):
    nc = tc.nc
    f32 = mybir.dt.float32
    f16 = mybir.dt.float16
    f8 = mybir.dt.float8e4
    f8w = mybir.dt.float8e5

    KT = K // P            # 32 k tiles
    KT16 = KT - nk8        # fp16 k tiles
    NPAIR = nk8 // 2       # fp8 DoubleRow pairs
    OT = O // P            # 16 o tiles
    NTCH = T // TCH        # token chunks
    OB = 2                 # o tiles per psum group (steady state)
    NOB = OT // OB
    # x16 arrives in three parts: 2 k-tiles (first MMs' dependency),
    # then the rest split evenly.
    XS = (2, (KT16 - 2 + 1) // 2, (KT16 - 2) // 2) if KT16 > 2 else (KT16, 0, 0)

    mult = mybir.AluOpType.mult
    addop = mybir.AluOpType.add

    with (
        tc.tile_pool(name="const", bufs=1) as const_pool,
        tc.tile_pool(name="wstage", bufs=SW) as wstage,
        tc.tile_pool(name="astage", bufs=3) as astage,
        tc.tile_pool(name="swt", bufs=1) as swt_pool,
        tc.tile_pool(name="xt", bufs=2) as xt_pool,
        tc.tile_pool(name="out", bufs=8) as out_pool,
        tc.tile_pool(name="psum_mm", bufs=8, space="PSUM") as psum_mm,
    ):
        scale_sb = const_pool.tile([P, OT], f32)
        bias_sb = const_pool.tile([P, OT], f32)
        partials = const_pool.tile([P, 2], f32)

        swt8 = swt_pool.tile([P, max(nk8, 1), O], f8)
        swt16 = swt_pool.tile([P, max(KT16, 1), O], f16)

        # ---- x chunks: scalar HWDGE ring (c0/c1 issued before the sign
        # ops take over the scalar queue) ----
        def load_x(c):
            parts = []
            k0 = 0
            for pi, nk in enumerate(XS):
                if nk == 0:
                    continue
                t = xt_pool.tile(
                    [P, nk, TCH], f16, tag=f"x16{pi}", name=f"x16{pi}_{c}"
                )
                nc.scalar.dma_start(t, xt16[c][:, ds(k0, nk), :])
                parts.append((k0, nk, t))
                k0 += nk
            t8 = None
            if nk8:
                t8 = xt_pool.tile([P, nk8, TCH], f8, tag="x8", name=f"x8_{c}")
                nc.scalar.dma_start(t8, xt8[c])
            return (parts, t8)

        def rhs16(xp, kt):
            for k0, nk, t in xp[0]:
                if kt < k0 + nk:
                    return t[:, kt - k0, :]
            raise AssertionError

        x_pre = {0: load_x(0), 1: load_x(1)}

        # ---- sign path: full-width contiguous fp8e5 slabs -> ACT sign in
        # two o-halves; all oh0 halves are produced before oh1 halves of
        # the stage window permits ----
        OH = O // 2
        kt_order = list(range(nk8, KT)) + list(range(nk8))
        staged = []

        def sign_half(kt, oh, slab):
            if kt < nk8:
                nc.scalar.sign(swt8[:, kt, ds(oh * OH, OH)], slab[:, ds(oh * OH, OH)])
            else:
                nc.scalar.sign(
                    swt16[:, kt - nk8, ds(oh * OH, OH)], slab[:, ds(oh * OH, OH)]
                )

        for i, kt in enumerate(kt_order):
            if i >= SW:
                okt, oslab = staged[i - SW]
                sign_half(okt, 1, oslab)  # free the ring slot
            slab = wstage.tile([P, O], f8w, tag="ws", name=f"ws_{kt}")
            nc.sync.dma_start(slab, wt[ds(kt * P, P), :])
            staged.append((kt, slab))
            sign_half(kt, 0, slab)
        for i in range(max(0, KT - SW), KT):
            okt, oslab = staged[i]
            sign_half(okt, 1, oslab)

        # ---- scale path: o-major f16 on the sync ring, behind the slabs;
        # the astage ring paces these DMAs against the DVE reduces, which
        # only stalls the (otherwise idle) sync queue ----
        wa_tiles = []
        for ot in range(OT):
            nc.sync.dma_start(
                bias_sb[:, ds(ot, 1)],
                b[ds(ot * P, P)].rearrange("(p one) -> p one", one=1),
            )
            halves = []
            for kh in range(2):
                wa = astage.tile([P, K // 2], f16, tag=f"wa{kh}", name=f"wa{kh}_{ot}")
                nc.sync.dma_start(wa, w16[ds(ot * P, P), ds(kh * K // 2, K // 2)])
                halves.append(wa)
            wa_tiles.append(halves)

        # ---- scale path compute on DVE ----
        for ot in range(OT):
            for kh in range(2):
                nc.vector.tensor_reduce(
                    out=partials[:, ds(kh, 1)],
                    in_=wa_tiles[ot][kh],
                    axis=mybir.AxisListType.X,
                    op=addop,
                    apply_absolute_value=True,
                )
            stot = const_pool.tile([P, 1], f32, tag="stot")
            nc.vector.tensor_reduce(
                out=stot, in_=partials, axis=mybir.AxisListType.X, op=addop
            )
            nc.vector.tensor_scalar_mul(scale_sb[:, ds(ot, 1)], stot, 1.0 / K)

        def evict(psum, ot, c, dmaq):
            out_sb = out_pool.tile([P, TCH], f16, name="osb")
            nc.vector.tensor_scalar(
                out_sb,
                psum,
                scale_sb[:, ds(ot, 1)],
                bias_sb[:, ds(ot, 1)],
                mult,
                addop,
            )
            dmaq.dma_start(yt[ds(ot * P, P), ds(c * TCH, TCH)], out_sb)

        def mm_group(psums, ots, xp):
            n_units = KT16 + NPAIR
            u = 0
            for kt in range(KT16):
                for psum, ot in zip(psums, ots):
                    nc.tensor.matmul(
                        psum,
                        lhsT=swt16[:, kt, ds(ot * P, P)],
                        rhs=rhs16(xp, kt),
                        start=(u == 0),
                        stop=(u == n_units - 1),
                    )
                u += 1
            for pr in range(NPAIR):
                for psum, ot in zip(psums, ots):
                    nc.tensor.matmul(
                        psum,
                        lhsT=swt8[:, ds(2 * pr, 2), ds(ot * P, P)],
                        rhs=xp[1][:, ds(2 * pr, 2), :],
                        start=(u == 0),
                        stop=(u == n_units - 1),
                        perf_mode=mybir.MatmulPerfMode.DoubleRow,
                    )
                u += 1

        # ---- main loop over token chunks ----
        for c in range(NTCH):
            xp = x_pre.pop(c)
            if c + 1 < NTCH and c + 1 not in x_pre:
                x_pre[c + 1] = load_x(c + 1)
            last = c == NTCH - 1
            if c == 0:
                # 8-o-tile groups across all psum banks: halves the rate
                # at which fresh sign slabs are consumed during warmup.
                groups = [list(range(8)), list(range(8, 16))]
            else:
                groups = [
                    list(range(ob * OB, (ob + 1) * OB)) for ob in range(NOB)
                ]
            for gi, ots in enumerate(groups):
                psums = [
                    psum_mm.tile([P, TCH], f32, tag="acc", name=f"acc{i}")
                    for i in range(len(ots))
                ]
                mm_group(psums, ots, xp)
                for i, (psum, ot) in enumerate(zip(psums, ots)):
                    if last:
                        dmaq = nc.sync if i % 2 == 0 else nc.scalar
                    else:
                        dmaq = nc.gpsimd
                    evict(psum, ot, c, dmaq)


def build_bass(T=T_FULL, K=K_FULL, O=O_FULL // N_CORES, nk8=NK8):
    nc = bacc.Bacc(trn_type="TRN2")
    f32 = mybir.dt.float32
    f16 = mybir.dt.float16
    f8 = mybir.dt.float8e4
    f8w = mybir.dt.float8e5
    KT16 = K // P - nk8
    NTCH = T // TCH
    xt8 = (
        nc.dram_tensor("xt8", [NTCH, P, nk8, TCH], f8, kind="ExternalInput").ap()
        if nk8
        else None
    )
    xt16 = (
        nc.dram_tensor("xt16", [NTCH, P, KT16, TCH], f16, kind="ExternalInput").ap()
        if KT16
        else None
    )
    wt = nc.dram_tensor("wt", [K, O], f8w, kind="ExternalInput").ap()
    w16 = nc.dram_tensor("w16", [O, K], f16, kind="ExternalInput").ap()
    b = nc.dram_tensor("b", [O], f32, kind="ExternalInput").ap()
    yt = nc.dram_tensor("yt", [O, T], f16, kind="ExternalOutput").ap()
    with tile.TileContext(nc) as tc:
        build_kernel_body(tc, xt8, xt16, wt, w16, b, yt, T, K, O, nk8)
    nc.finalize()
    return nc


_CACHED_NC = None


def _get_nc():
    global _CACHED_NC
    if _CACHED_NC is None:
        _CACHED_NC = build_bass()
    return _CACHED_NC


def make_in_maps(x, weight, bias):
    x = np.asarray(x, dtype=np.float32)
    weight = np.ascontiguousarray(np.asarray(weight, dtype=np.float32))
    bias = np.ascontiguousarray(np.asarray(bias, dtype=np.float32))
    O = weight.shape[0] // N_CORES
    K = x.shape[1]
    T = x.shape[0]
    KT16 = K // P - NK8
    # host-side marshaling: transpose + cast + tile layout
    xt = np.ascontiguousarray(x.T)  # [K, T]
    base = {}
    NTCH = T // TCH
    if NK8:
        base["xt8"] = np.ascontiguousarray(
            xt[: NK8 * P].reshape(NK8, P, NTCH, TCH).transpose(2, 1, 0, 3)
        ).astype(ml_dtypes.float8_e4m3fn)
    if KT16:
        base["xt16"] = np.ascontiguousarray(
            xt[NK8 * P :].reshape(KT16, P, NTCH, TCH).transpose(2, 1, 0, 3)
        ).astype(np.float16)
    # sign path: k-major fp8e5; values that would land in the denormal/
    # zero range are pinned to +-2^-14 (min normal) so sign() on device
    # is exactly sign(W).
    wtf = np.ascontiguousarray(weight.T)  # [K, O_FULL] f32
    wt8 = wtf.astype(ml_dtypes.float8_e5m2)
    tiny = np.abs(wtf) < np.float32(2.0**-14)
    if tiny.any():
        fix = np.copysign(np.float32(2.0**-14), wtf).astype(ml_dtypes.float8_e5m2)
        wt8 = np.where(tiny, fix, wt8)
    # scale path: o-major fp16
    w16 = weight.astype(np.float16)  # [O_FULL, K]
    return [
        {
            **base,
            "wt": np.ascontiguousarray(wt8[:, c * O : (c + 1) * O]),
            "w16": np.ascontiguousarray(w16[c * O : (c + 1) * O]),
            "b": bias[c * O : (c + 1) * O],
        }
        for c in range(N_CORES)
    ]


def kernel(x, weight, bias):
    nc = _get_nc()
    in_maps = make_in_maps(x, weight, bias)
    res = run_bass_kernel_spmd(nc, in_maps, list(range(N_CORES)))
    yt = np.concatenate([r["yt"] for r in res.results], axis=0)  # [O_FULL, T] f16
    return np.ascontiguousarray(yt.T.astype(np.float32))


# revision 14
# speedup vs baseline: 1.0067x; 1.0067x over previous
"""Trainium2 Bass kernel for MockBitNetLayer:

    scale = mean(|W|, axis=1)            # [O, 1]
    y = x @ (sign(W) * scale).T + bias   # [T, O]

Strategy (column-parallel over 8 NeuronCores), v6:
  - Each core owns an O/8 = 2048-column shard of W.T and bias; x is
    shared.  Host-side input marshaling (transpose + dtype cast + tile
    layout) is done in numpy during sharding; all model arithmetic
    (sign, |W| mean, matmul, scale/bias) runs on device.
  - Precision split over the contraction: the first NK8*128 rows of x
    in fp8e4 (consumed by DoubleRow MMs, 2 k-tiles per 216 ns slot),
    the rest in fp16 (1 k-tile per slot).  NK8=18 measures 1.989e-2
    against the fp32 reference (tolerance 2e-2), predicted exactly by
    a host-side numpy simulation of the quantization chain.
  - W arrives twice: k-major fp8e5 (sign path; values that would round
    to zero/denormal are host-pinned to +-2^-14 so sign() is exact) and
    o-major fp16 (scale path).
  - Engine/queue split (v5 traces showed sign-slab DMAs pacing the
    whole warmup, and the gpsimd SWDGE ring sustaining only ~80 GB/s):
      sync   : 32 full-width contiguous W.T slab DMAs (256 KB each),
               then the o-major W + bias for the scale path
      scalar : 64 sign ops (two 1024-wide halves per slab, oh0 halves
               first so the first MM blocks' weights appear earliest),
               y write-out DMA triggers
      vector : |W| row reduces (f16, 2x DVE rate) + 1/K, then all psum
               evictions (tensor_scalar psum*scale+bias -> f16)
      gpsimd : x chunk DMAs (slow SWDGE ring, but nothing here is
               latency-critical and it keeps the HWDGE rings clear)
      tensor : nothing but the MM stream
  - Chunk 0 runs (ob0..3)/(ob4..7) interleaved across 8 PSUM banks,
    halving the fresh-slab consumption rate while the sign pipeline
    warms; later chunks roll 2-o-tile psum groups (4 in flight).
  - y is written as f16 (halves output traffic); the last chunk's y
    DMAs go out on the idle sync/scalar rings to shrink the tail.
"""

import os
import sys

for _p in ("/opt/trn_rl_repo", "/root/.axon_site/_ro/trn_rl_repo"):
    if os.path.isdir(_p) and _p not in sys.path:
        sys.path.insert(0, _p)

import numpy as np
import ml_dtypes

import concourse.bacc as bacc
import concourse.mybir as mybir
import concourse.tile as tile
from concourse.bass import ds
from concourse.bass_utils import run_bass_kernel_spmd

P = 128
N_CORES = 8

T_FULL = 8192
K_FULL = 4096
O_FULL = 16384

NK8 = int(os.environ.get("NK8T", "18"))  # fp8 k-tiles (even, 0..32)
TCH = 512
SW = 24  # W.T slab stage ring depth


def build_kernel_body(tc, xt8, xt16, wt, w16, b, yt, T, K, O, nk8):
    nc = tc.nc
    f32 = mybir.dt.float32
    f16 = mybir.dt.float16
    f8 = mybir.dt.float8e4
    f8w = mybir.dt.float8e5

    KT = K // P            # 32 k tiles
    KT16 = KT - nk8        # fp16 k tiles
    NPAIR = nk8 // 2       # fp8 DoubleRow pairs
    OT = O // P            # 16 o tiles
    NTCH = T // TCH        # token chunks
    OB = 4                 # o tiles per psum group (steady state)
    NOB = OT // OB
    # x16 arrives in three parts: 2 k-tiles (first MMs' dependency),
    # then the rest split evenly.
    XS = (2, (KT16 - 2 + 1) // 2, (KT16 - 2) // 2) if KT16 > 2 else (KT16, 0, 0)

    mult = mybir.AluOpType.mult
    addop = mybir.AluOpType.add

    with (
        tc.tile_pool(name="const", bufs=1) as const_pool,
        tc.tile_pool(name="wstage", bufs=SW) as wstage,
        tc.tile_pool(name="astage", bufs=2) as astage,
        tc.tile_pool(name="swt", bufs=1) as swt_pool,
        tc.tile_pool(name="xt", bufs=2) as xt_pool,
        tc.tile_pool(name="out", bufs=4) as out_pool,
        tc.tile_pool(name="psum_mm", bufs=8, space="PSUM") as psum_mm,
    ):
        scale_sb = const_pool.tile([P, OT], f32)
        bias_sb = const_pool.tile([P, OT], f32)
        partials = const_pool.tile([P, 2], f32)

        swt8 = swt_pool.tile([P, max(nk8, 1), O], f8)
        swt16 = swt_pool.tile([P, max(KT16, 1), O], f16)

        # ---- x chunks: gpsimd SWDGE ring (background-rate, interferes
        # with nothing latency-critical) ----
        def load_x(c):
            parts = []
            k0 = 0
            for pi, nk in enumerate(XS):
                if nk == 0:
                    continue
                t = xt_pool.tile(
                    [P, nk, TCH], f16, tag=f"x16{pi}", name=f"x16{pi}_{c}"
                )
                nc.gpsimd.dma_start(t, xt16[c][:, ds(k0, nk), :])
                parts.append((k0, nk, t))
                k0 += nk
            t8 = None
            if nk8:
                t8 = xt_pool.tile([P, nk8, TCH], f8, tag="x8", name=f"x8_{c}")
                nc.gpsimd.dma_start(t8, xt8[c])
            return (parts, t8)

        def rhs16(xp, kt):
            for k0, nk, t in xp[0]:
                if kt < k0 + nk:
                    return t[:, kt - k0, :]
            raise AssertionError

        x_pre = {0: load_x(0), 1: load_x(1)}

        # ---- sign path: full-width contiguous fp8e5 slabs -> ACT sign in
        # two o-halves; all oh0 halves are produced before oh1 halves of
        # the stage window permits ----
        OH = O // 2
        kt_order = list(range(nk8, KT)) + list(range(nk8))
        staged = []

        def sign_half(kt, oh, slab):
            if kt < nk8:
                nc.scalar.sign(swt8[:, kt, ds(oh * OH, OH)], slab[:, ds(oh * OH, OH)])
            else:
                nc.scalar.sign(
                    swt16[:, kt - nk8, ds(oh * OH, OH)], slab[:, ds(oh * OH, OH)]
                )

        for i, kt in enumerate(kt_order):
            if i >= SW:
                okt, oslab = staged[i - SW]
                sign_half(okt, 1, oslab)  # free the ring slot
            slab = wstage.tile([P, O], f8w, tag="ws", name=f"ws_{kt}")
            nc.sync.dma_start(slab, wt[ds(kt * P, P), :])
            staged.append((kt, slab))
            sign_half(kt, 0, slab)
        for i in range(max(0, KT - SW), KT):
            okt, oslab = staged[i]
            sign_half(okt, 1, oslab)

        # ---- scale path: o-major f16 on the sync ring, behind the slabs;
        # the astage ring paces these DMAs against the DVE reduces, which
        # only stalls the (otherwise idle) sync queue ----
        wa_tiles = []
        for ot in range(OT):
            nc.sync.dma_start(
                bias_sb[:, ds(ot, 1)],
                b[ds(ot * P, P)].rearrange("(p one) -> p one", one=1),
            )
            halves = []
            for kh in range(2):
                wa = astage.tile([P, K // 2], f16, tag=f"wa{kh}", name=f"wa{kh}_{ot}")
                nc.sync.dma_start(wa, w16[ds(ot * P, P), ds(kh * K // 2, K // 2)])
                halves.append(wa)
            wa_tiles.append(halves)

        # ---- scale path compute on DVE ----
        for ot in range(OT):
            for kh in range(2):
                nc.vector.tensor_reduce(
                    out=partials[:, ds(kh, 1)],
                    in_=wa_tiles[ot][kh],
                    axis=mybir.AxisListType.X,
                    op=addop,
                    apply_absolute_value=True,
                )
            stot = const_pool.tile([P, 1], f32, tag="stot")
            nc.vector.tensor_reduce(
                out=stot, in_=partials, axis=mybir.AxisListType.X, op=addop
            )
            nc.vector.tensor_scalar_mul(scale_sb[:, ds(ot, 1)], stot, 1.0 / K)

        def evict(psum, ot, c, dmaq):
            out_sb = out_pool.tile([P, TCH], f16, name="osb")
            nc.vector.tensor_scalar(
                out_sb,
                psum,
                scale_sb[:, ds(ot, 1)],
                bias_sb[:, ds(ot, 1)],
                mult,
                addop,
            )
            dmaq.dma_start(yt[ds(ot * P, P), ds(c * TCH, TCH)], out_sb)

        def mm_group(psums, ots, xp):
            n_units = KT16 + NPAIR
            u = 0
            for kt in range(KT16):
                for psum, ot in zip(psums, ots):
                    nc.tensor.matmul(
                        psum,
                        lhsT=swt16[:, kt, ds(ot * P, P)],
                        rhs=rhs16(xp, kt),
                        start=(u == 0),
                        stop=(u == n_units - 1),
                    )
                u += 1
            for pr in range(NPAIR):
                for psum, ot in zip(psums, ots):
                    nc.tensor.matmul(
                        psum,
                        lhsT=swt8[:, ds(2 * pr, 2), ds(ot * P, P)],
                        rhs=xp[1][:, ds(2 * pr, 2), :],
                        start=(u == 0),
                        stop=(u == n_units - 1),
                        perf_mode=mybir.MatmulPerfMode.DoubleRow,
                    )
                u += 1

        # ---- main loop over token chunks ----
        for c in range(NTCH):
            xp = x_pre.pop(c)
            if c + 1 < NTCH and c + 1 not in x_pre:
                x_pre[c + 1] = load_x(c + 1)
            last = c == NTCH - 1
            if c == 0:
                # 8-o-tile groups across all psum banks: halves the rate
                # at which fresh sign slabs are consumed during warmup.
                groups = [list(range(8)), list(range(8, 16))]
            else:
                groups = [
                    list(range(ob * OB, (ob + 1) * OB)) for ob in range(NOB)
                ]
            for gi, ots in enumerate(groups):
                psums = [
                    psum_mm.tile([P, TCH], f32, tag="acc", name=f"acc{i}")
                    for i in range(len(ots))
                ]
                mm_group(psums, ots, xp)
                for i, (psum, ot) in enumerate(zip(psums, ots)):
                    if last:
                        dmaq = nc.sync if i % 2 == 0 else nc.scalar
                    else:
                        dmaq = nc.scalar
                    evict(psum, ot, c, dmaq)


def build_bass(T=T_FULL, K=K_FULL, O=O_FULL // N_CORES, nk8=NK8):
    nc = bacc.Bacc(trn_type="TRN2")
    f32 = mybir.dt.float32
    f16 = mybir.dt.float16
    f8 = mybir.dt.float8e4
    f8w = mybir.dt.float8e5
    KT16 = K // P - nk8
    NTCH = T // TCH
    xt8 = (
        nc.dram_tensor("xt8", [NTCH, P, nk8, TCH], f8, kind="ExternalInput").ap()
        if nk8
        else None
    )
    xt16 = (
        nc.dram_tensor("xt16", [NTCH, P, KT16, TCH], f16, kind="ExternalInput").ap()
        if KT16
        else None
    )
    wt = nc.dram_tensor("wt", [K, O], f8w, kind="ExternalInput").ap()
    w16 = nc.dram_tensor("w16", [O, K], f16, kind="ExternalInput").ap()
    b = nc.dram_tensor("b", [O], f32, kind="ExternalInput").ap()
    yt = nc.dram_tensor("yt", [O, T], f16, kind="ExternalOutput").ap()
    with tile.TileContext(nc) as tc:
        build_kernel_body(tc, xt8, xt16, wt, w16, b, yt, T, K, O, nk8)
    nc.finalize()
    return nc


_CACHED_NC = None


def _get_nc():
    global _CACHED_NC
    if _CACHED_NC is None:
        _CACHED_NC = build_bass()
    return _CACHED_NC


def make_in_maps(x, weight, bias):
    x = np.asarray(x, dtype=np.float32)
    weight = np.ascontiguousarray(np.asarray(weight, dtype=np.float32))
    bias = np.ascontiguousarray(np.asarray(bias, dtype=np.float32))
    O = weight.shape[0] // N_CORES
    K = x.shape[1]
    T = x.shape[0]
    KT16 = K // P - NK8
    # host-side marshaling: transpose + cast + tile layout
    xt = np.ascontiguousarray(x.T)  # [K, T]
    base = {}
    NTCH = T // TCH
    if NK8:
        base["xt8"] = np.ascontiguousarray(
            xt[: NK8 * P].reshape(NK8, P, NTCH, TCH).transpose(2, 1, 0, 3)
        ).astype(ml_dtypes.float8_e4m3fn)
    if KT16:
        base["xt16"] = np.ascontiguousarray(
            xt[NK8 * P :].reshape(KT16, P, NTCH, TCH).transpose(2, 1, 0, 3)
        ).astype(np.float16)
    # sign path: k-major fp8e5; values that would land in the denormal/
    # zero range are pinned to +-2^-14 (min normal) so sign() on device
    # is exactly sign(W).
    wtf = np.ascontiguousarray(weight.T)  # [K, O_FULL] f32
    wt8 = wtf.astype(ml_dtypes.float8_e5m2)
    tiny = np.abs(wtf) < np.float32(2.0**-14)
    if tiny.any():
        fix = np.copysign(np.float32(2.0**-14), wtf).astype(ml_dtypes.float8_e5m2)
        wt8 = np.where(tiny, fix, wt8)
    # scale path: o-major fp16
    w16 = weight.astype(np.float16)  # [O_FULL, K]
    return [
        {
            **base,
            "wt": np.ascontiguousarray(wt8[:, c * O : (c + 1) * O]),
            "w16": np.ascontiguousarray(w16[c * O : (c + 1) * O]),
            "b": bias[c * O : (c + 1) * O],
        }
        for c in range(N_CORES)
    ]


def kernel(x, weight, bias):
    nc = _get_nc()
    in_maps = make_in_maps(x, weight, bias)
    res = run_bass_kernel_spmd(nc, in_maps, list(range(N_CORES)))
    yt = np.concatenate([r["yt"] for r in res.results], axis=0)  # [O_FULL, T] f16
    return np.ascontiguousarray(yt.T.astype(np.float32))


# revision 18
# speedup vs baseline: 1.0130x; 1.0063x over previous
"""Trainium2 Bass kernel for MockBitNetLayer:

    scale = mean(|W|, axis=1)            # [O, 1]
    y = x @ (sign(W) * scale).T + bias   # [T, O]

Strategy (column-parallel over 8 NeuronCores), v6:
  - Each core owns an O/8 = 2048-column shard of W.T and bias; x is
    shared.  Host-side input marshaling (transpose + dtype cast + tile
    layout) is done in numpy during sharding; all model arithmetic
    (sign, |W| mean, matmul, scale/bias) runs on device.
  - Precision split over the contraction: the first NK8*128 rows of x
    in fp8e4 (consumed by DoubleRow MMs, 2 k-tiles per 216 ns slot),
    the rest in fp16 (1 k-tile per slot).  NK8=18 measures 1.989e-2
    against the fp32 reference (tolerance 2e-2), predicted exactly by
    a host-side numpy simulation of the quantization chain.
  - W arrives twice: k-major fp8e5 (sign path; values that would round
    to zero/denormal are host-pinned to +-2^-14 so sign() is exact) and
    o-major fp16 (scale path).
  - Engine/queue split (v5 traces showed sign-slab DMAs pacing the
    whole warmup, and the gpsimd SWDGE ring sustaining only ~80 GB/s):
      sync   : 32 full-width contiguous W.T slab DMAs (256 KB each),
               then the o-major W + bias for the scale path
      scalar : 64 sign ops (two 1024-wide halves per slab, oh0 halves
               first so the first MM blocks' weights appear earliest),
               y write-out DMA triggers
      vector : |W| row reduces (f16, 2x DVE rate) + 1/K, then all psum
               evictions (tensor_scalar psum*scale+bias -> f16)
      gpsimd : x chunk DMAs (slow SWDGE ring, but nothing here is
               latency-critical and it keeps the HWDGE rings clear)
      tensor : nothing but the MM stream
  - Chunk 0 runs (ob0..3)/(ob4..7) interleaved across 8 PSUM banks,
    halving the fresh-slab consumption rate while the sign pipeline
    warms; later chunks roll 2-o-tile psum groups (4 in flight).
  - y is written as f16 (halves output traffic); the last chunk's y
    DMAs go out on the idle sync/scalar rings to shrink the tail.
"""

import os
import sys

for _p in ("/opt/trn_rl_repo", "/root/.axon_site/_ro/trn_rl_repo"):
    if os.path.isdir(_p) and _p not in sys.path:
        sys.path.insert(0, _p)

import numpy as np
import ml_dtypes

import concourse.bacc as bacc
import concourse.mybir as mybir
import concourse.tile as tile
from concourse.bass import ds
from concourse.bass_utils import run_bass_kernel_spmd

P = 128
N_CORES = 8

T_FULL = 8192
K_FULL = 4096
O_FULL = 16384

NK8 = int(os.environ.get("NK8T", "18"))  # fp8 k-tiles (even, 0..32)
TCH = 512


def build_kernel_body(tc, xt8, xt16, wt, w16, b, yt, T, K, O, nk8):
    nc = tc.nc
    f32 = mybir.dt.float32
    f16 = mybir.dt.float16
    f8 = mybir.dt.float8e4
    f8w = mybir.dt.float8e5

    KT = K // P            # 32 k tiles
    KT16 = KT - nk8        # fp16 k tiles
    NPAIR = nk8 // 2       # fp8 DoubleRow pairs
    OT = O // P            # 16 o tiles
    NTCH = T // TCH        # token chunks
    OB = 4                 # o tiles per psum group (steady state)
    NOB = OT // OB
    # x16 arrives in three parts: 2 k-tiles (first MMs' dependency),
    # then the rest split evenly.
    XS = (2, (KT16 - 2 + 1) // 2, (KT16 - 2) // 2) if KT16 > 2 else (KT16, 0, 0)

    mult = mybir.AluOpType.mult
    addop = mybir.AluOpType.add

    with (
        tc.tile_pool(name="const", bufs=1) as const_pool,
        tc.tile_pool(name="wstage", bufs=2) as wstage,
        tc.tile_pool(name="astage", bufs=2) as astage,
        tc.tile_pool(name="swt", bufs=1) as swt_pool,
        tc.tile_pool(name="xt", bufs=2) as xt_pool,
        tc.tile_pool(name="out", bufs=8) as out_pool,
        tc.tile_pool(name="psum_mm", bufs=8, space="PSUM") as psum_mm,
    ):
        scale_sb = const_pool.tile([P, OT], f32)
        bias_sb = const_pool.tile([P, OT], f32)
        partials = const_pool.tile([P, 2], f32)

        swt8 = swt_pool.tile([P, max(nk8, 1), O], f8)
        swt16 = swt_pool.tile([P, max(KT16, 1), O], f16)

        # ---- x chunks: gpsimd SWDGE ring (background-rate, interferes
        # with nothing latency-critical) ----
        def load_x(c):
            parts = []
            k0 = 0
            for pi, nk in enumerate(XS):
                if nk == 0:
                    continue
                t = xt_pool.tile(
                    [P, nk, TCH], f16, tag=f"x16{pi}", name=f"x16{pi}_{c}"
                )
                q = nc.sync if (c == 0 and pi == 0) else nc.gpsimd
                q.dma_start(t, xt16[c][:, ds(k0, nk), :])
                parts.append((k0, nk, t))
                k0 += nk
            t8 = None
            if nk8:
                t8 = xt_pool.tile([P, nk8, TCH], f8, tag="x8", name=f"x8_{c}")
                nc.gpsimd.dma_start(t8, xt8[c])
            return (parts, t8)

        def rhs16(xp, kt):
            for k0, nk, t in xp[0]:
                if kt < k0 + nk:
                    return t[:, kt - k0, :]
            raise AssertionError

        x_pre = {0: load_x(0), 1: load_x(1)}

        # ---- sign path.  Per-DMA fixed cost (~1.5 us) dominates small
        # transfers, so slabs are fetched four k-tiles per DMA via a
        # rearranged AP.  Pass A signs the oh0 halves of every k-tile
        # (all of chunk 0's first psum group) at the ACT engine's rate;
        # pass B re-fetches the oh1 halves (quad-batched, strided) and
        # signs them.  Order inside each pass is MM consumption order
        # (fp16 k-tiles, then fp8). ----
        OH = O // 2
        kt_order = list(range(nk8, KT)) + list(range(nk8))
        quads = []
        for seg in (kt_order[:KT16], kt_order[KT16:]):
            for j in range(0, len(seg), 4):
                quads.append(seg[j : j + 4])

        def sign_dst(kt, oh):
            if kt < nk8:
                return swt8[:, kt, ds(oh * OH, OH)]
            return swt16[:, kt - nk8, ds(oh * OH, OH)]

        for q in quads:
            gs = len(q)
            slab = wstage.tile([P, gs, O], f8w, tag=f"wsa{gs}", name=f"wsa_{q[0]}")
            nc.sync.dma_start(
                slab, wt[ds(q[0], gs), :, :].rearrange("g p o -> p g o")
            )
            for gi, kt in enumerate(q):
                nc.scalar.sign(sign_dst(kt, 0), slab[:, gi, ds(0, OH)])
        for q in quads:
            gs = len(q)
            slab = wstage.tile([P, gs, OH], f8w, tag=f"wsb{gs}", name=f"wsb_{q[0]}")
            nc.sync.dma_start(
                slab,
                wt[ds(q[0], gs), :, ds(OH, OH)].rearrange("g p o -> p g o"),
            )
            for gi, kt in enumerate(q):
                nc.scalar.sign(sign_dst(kt, 1), slab[:, gi, :])

        # ---- scale path: o-major f16 on the sync ring, behind the slabs;
        # the astage ring paces these DMAs against the DVE reduces, which
        # only stalls the (otherwise idle) sync queue ----
        wa_tiles = []
        for ot in range(OT):
            nc.sync.dma_start(
                bias_sb[:, ds(ot, 1)],
                b[ds(ot * P, P)].rearrange("(p one) -> p one", one=1),
            )
            halves = []
            for kh in range(2):
                wa = astage.tile([P, K // 2], f16, tag=f"wa{kh}", name=f"wa{kh}_{ot}")
                nc.sync.dma_start(wa, w16[ds(ot * P, P), ds(kh * K // 2, K // 2)])
                halves.append(wa)
            wa_tiles.append(halves)

        # ---- scale path compute on DVE ----
        for ot in range(OT):
            for kh in range(2):
                nc.vector.tensor_reduce(
                    out=partials[:, ds(kh, 1)],
                    in_=wa_tiles[ot][kh],
                    axis=mybir.AxisListType.X,
                    op=addop,
                    apply_absolute_value=True,
                )
            stot = const_pool.tile([P, 1], f32, tag="stot")
            nc.vector.tensor_reduce(
                out=stot, in_=partials, axis=mybir.AxisListType.X, op=addop
            )
            nc.vector.tensor_scalar_mul(scale_sb[:, ds(ot, 1)], stot, 1.0 / K)

        def evict(psum, ot, c, dmaq):
            out_sb = out_pool.tile([P, TCH], f16, name="osb")
            nc.vector.tensor_scalar(
                out_sb,
                psum,
                scale_sb[:, ds(ot, 1)],
                bias_sb[:, ds(ot, 1)],
                mult,
                addop,
            )
            dmaq.dma_start(yt[ds(ot * P, P), ds(c * TCH, TCH)], out_sb)

        def mm_group(psums, ots, xp):
            n_units = KT16 + NPAIR
            u = 0
            for kt in range(KT16):
                for psum, ot in zip(psums, ots):
                    nc.tensor.matmul(
                        psum,
                        lhsT=swt16[:, kt, ds(ot * P, P)],
                        rhs=rhs16(xp, kt),
                        start=(u == 0),
                        stop=(u == n_units - 1),
                    )
                u += 1
            for pr in range(NPAIR):
                for psum, ot in zip(psums, ots):
                    nc.tensor.matmul(
                        psum,
                        lhsT=swt8[:, ds(2 * pr, 2), ds(ot * P, P)],
                        rhs=xp[1][:, ds(2 * pr, 2), :],
                        start=(u == 0),
                        stop=(u == n_units - 1),
                        perf_mode=mybir.MatmulPerfMode.DoubleRow,
                    )
                u += 1

        # ---- main loop over token chunks ----
        for c in range(NTCH):
            xp = x_pre.pop(c)
            if c + 1 < NTCH and c + 1 not in x_pre:
                x_pre[c + 1] = load_x(c + 1)
            last = c == NTCH - 1
            if c == 0:
                # 8-o-tile groups across all psum banks: halves the rate
                # at which fresh sign slabs are consumed during warmup.
                groups = [list(range(8)), list(range(8, 16))]
            else:
                groups = [
                    list(range(ob * OB, (ob + 1) * OB)) for ob in range(NOB)
                ]
            for gi, ots in enumerate(groups):
                psums = [
                    psum_mm.tile([P, TCH], f32, tag="acc", name=f"acc{i}")
                    for i in range(len(ots))
                ]
                mm_group(psums, ots, xp)
                for i, (psum, ot) in enumerate(zip(psums, ots)):
                    if last:
                        dmaq = nc.sync if i % 2 == 0 else nc.scalar
                    else:
                        dmaq = nc.scalar
                    evict(psum, ot, c, dmaq)


def build_bass(T=T_FULL, K=K_FULL, O=O_FULL // N_CORES, nk8=NK8):
    nc = bacc.Bacc(trn_type="TRN2")
    f32 = mybir.dt.float32
    f16 = mybir.dt.float16
    f8 = mybir.dt.float8e4
    f8w = mybir.dt.float8e5
    KT16 = K // P - nk8
    NTCH = T // TCH
    xt8 = (
        nc.dram_tensor("xt8", [NTCH, P, nk8, TCH], f8, kind="ExternalInput").ap()
        if nk8
        else None
    )
    xt16 = (
        nc.dram_tensor("xt16", [NTCH, P, KT16, TCH], f16, kind="ExternalInput").ap()
        if KT16
        else None
    )
    wt = nc.dram_tensor("wt", [K // P, P, O], f8w, kind="ExternalInput").ap()
    w16 = nc.dram_tensor("w16", [O, K], f16, kind="ExternalInput").ap()
    b = nc.dram_tensor("b", [O], f32, kind="ExternalInput").ap()
    yt = nc.dram_tensor("yt", [O, T], f16, kind="ExternalOutput").ap()
    with tile.TileContext(nc) as tc:
        build_kernel_body(tc, xt8, xt16, wt, w16, b, yt, T, K, O, nk8)
    nc.finalize()
    return nc


_CACHED_NC = None


def _get_nc():
    global _CACHED_NC
    if _CACHED_NC is None:
        _CACHED_NC = build_bass()
    return _CACHED_NC


def make_in_maps(x, weight, bias):
    x = np.asarray(x, dtype=np.float32)
    weight = np.ascontiguousarray(np.asarray(weight, dtype=np.float32))
    bias = np.ascontiguousarray(np.asarray(bias, dtype=np.float32))
    O = weight.shape[0] // N_CORES
    K = x.shape[1]
    T = x.shape[0]
    KT16 = K // P - NK8
    # host-side marshaling: transpose + cast + tile layout
    xt = np.ascontiguousarray(x.T)  # [K, T]
    base = {}
    NTCH = T // TCH
    if NK8:
        base["xt8"] = np.ascontiguousarray(
            xt[: NK8 * P].reshape(NK8, P, NTCH, TCH).transpose(2, 1, 0, 3)
        ).astype(ml_dtypes.float8_e4m3fn)
    if KT16:
        base["xt16"] = np.ascontiguousarray(
            xt[NK8 * P :].reshape(KT16, P, NTCH, TCH).transpose(2, 1, 0, 3)
        ).astype(np.float16)
    # sign path: k-major fp8e5; values that would land in the denormal/
    # zero range are pinned to +-2^-14 (min normal) so sign() on device
    # is exactly sign(W).
    wtf = np.ascontiguousarray(weight.T)  # [K, O_FULL] f32
    wt8 = wtf.astype(ml_dtypes.float8_e5m2)
    tiny = np.abs(wtf) < np.float32(2.0**-14)
    if tiny.any():
        fix = np.copysign(np.float32(2.0**-14), wtf).astype(ml_dtypes.float8_e5m2)
        wt8 = np.where(tiny, fix, wt8)
    # scale path: o-major fp16
    w16 = weight.astype(np.float16)  # [O_FULL, K]
    return [
        {
            **base,
            "wt": np.ascontiguousarray(wt8[:, c * O : (c + 1) * O]),
            "w16": np.ascontiguousarray(w16[c * O : (c + 1) * O]),
            "b": bias[c * O : (c + 1) * O],
        }
        for c in range(N_CORES)
    ]


def kernel(x, weight, bias):
    nc = _get_nc()
    in_maps = make_in_maps(x, weight, bias)
    res = run_bass_kernel_spmd(nc, in_maps, list(range(N_CORES)))
    yt = np.concatenate([r["yt"] for r in res.results], axis=0)  # [O_FULL, T] f16
    return np.ascontiguousarray(yt.T.astype(np.float32))
